# revision 39
# baseline (speedup 1.0000x reference)
"""CFG dual cross-attention on 8 Trainium2 NeuronCores (Bass/Tile).

Sharding: the cfg axis (cond/uncond) splits the 8 cores into 2 groups of 4;
within a group the 4096 query rows are sharded 4-way (1024 rows/core) and the
K/V projection is sharded 4-way over heads.  Each core computes K^T/V for its
10 heads, the group AllGathers K/V (plus exact partial sum-of-squares rows for
the K rms-norm), and every core then runs all 40 heads of attention over its
own query rows.  The host concatenates the row shards.

Matmul operands are bf16 (fp32 PSUM accumulation); softmax/rms statistics in
fp32.  All weights are repacked host-side so every streamed weight tile is a
single fully-contiguous DMA read, and hT/cT are packed per-chunk contiguous.

Attention uses the transposed-logits formulation: logits [L-part, s-free] per
head, exp on the scalar engine, key-axis sum via ones-matmul, softmax 1/sum
via the fast approx reciprocal, replicated across partitions with a rank-1
f32r matmul issued *after* the A@V matmuls so the reciprocal latency hides
under PE work.  Per-m rms sum-of-squares matmuls are delayed by one m-tile so
the eviction->square chain never stalls the PE stream.
"""

from contextlib import ExitStack

import numpy as np

import concourse.bass as bass
import concourse.bacc as bacc
import concourse.mybir as mybir
import concourse.tile as tile
from concourse import bass_utils

EPS = 1e-6
F32 = mybir.dt.float32
F32R = mybir.dt.float32r

# ---- problem shape (nn_CFGDualCrossAttention: D=5120, H=40, S=4096, L=512) ----
D = 5120
L = 512
S_SHARD = 1024        # 4096 / 4 cores per cfg group
KO = D // 128         # contraction subtiles == heads (head_dim 128)
H = KO
LSUB = L // 128
QCH = 512             # q projection chunk (2 per shard)
SCH = 256             # attention sub-chunk (2 per q chunk)
NSUB = S_SHARD // SCH
R = 4                 # cores per cfg group
MSH = KO // R         # kv-shard m-tiles (10)
VSH = D // R          # kv-shard output cols (1280)
MM = mybir.dt.bfloat16
SCALE = float(128 ** -0.5)

# AllGather buffer layout (bf16 elements)
K_ELEMS = MSH * 128 * L           # 655360
SS_ELEMS = L                      # 512  (partial sum-of-squares row)
V_ELEMS = LSUB * 128 * VSH        # 655360
SHARD_ELEMS = K_ELEMS + SS_ELEMS + V_ELEMS

TRACE = False         # set by test harness for NTFF timing
LAST_EXEC_NS = None
_CACHED_NC = None


def _build() -> bacc.Bacc:
    mm = MM
    WKT = 5           # wk/wv stream tiles (256 cols each)
    WOT = D // 512    # wo stream tiles

    nc = bacc.Bacc("TRN2", target_bir_lowering=False, debug=False, num_devices=8)

    # ---- external inputs (host-side repacked; see kernel() below) ----
    hT_p = nc.dram_tensor("hT_p", [128, KO * S_SHARD], mm,
                          kind="ExternalInput")
    cT_p = nc.dram_tensor("cT_p", [128, KO * L], mm, kind="ExternalInput")
    wq_p = nc.dram_tensor("wq_p", [KO, 128, KO * 128], mm, kind="ExternalInput")
    wk_p = nc.dram_tensor("wk_p", [WKT, 128, KO * 256], mm, kind="ExternalInput")
    wv_p = nc.dram_tensor("wv_p", [WKT, 128, KO * 256], mm, kind="ExternalInput")
    wo_p = nc.dram_tensor("wo_p", [WOT, 128, KO * 512], mm, kind="ExternalInput")
    gq_pm = nc.dram_tensor("gq_pm", [128, KO], F32, kind="ExternalInput")
    bqgq_pm = nc.dram_tensor("bqgq_pm", [128, KO], F32, kind="ExternalInput")
    gk_pm = nc.dram_tensor("gk_pm", [128, MSH], F32, kind="ExternalInput")
    bkgk_pm = nc.dram_tensor("bkgk_pm", [128, MSH], F32, kind="ExternalInput")
    bv_sh = nc.dram_tensor("bv_sh", [VSH], F32, kind="ExternalInput")
    bot = nc.dram_tensor("bo", [D], F32, kind="ExternalInput")
    out = nc.dram_tensor("out", [S_SHARD, D], mm, kind="ExternalOutput")

    oT_dram = nc.dram_tensor("oT_spill", [D, S_SHARD], mm)
    qT_dram = nc.dram_tensor("qT_spill", [KO, 128, S_SHARD], mm)
    kv_in = nc.dram_tensor("kv_in", [SHARD_ELEMS], mm)
    # note: Shared addr_space needs >4-core groups; Local costs one extra copy
    kv_out = nc.dram_tensor("kv_out", [R * SHARD_ELEMS], mm)

    oT_r = oT_dram.rearrange("(ko p) s -> p ko s", p=128)
    out_r = out.rearrange("(cs p) n -> p cs n", p=128)

    replica_groups = [[0, 1, 2, 3], [4, 5, 6, 7]]

    def wdma(i, dst, src):
        # alternate big streaming DMAs across the two HWDGE queues
        (nc.sync if i % 2 == 0 else nc.scalar).dma_start(dst, src)

    def wdma2(dst, src):
        # split one weight tile across both HWDGE queues (halved latency)
        half = dst.shape[1] // 2
        nc.sync.dma_start(dst[:, :half], src[:, :half])
        nc.scalar.dma_start(dst[:, half:], src[:, half:])

    with tile.TileContext(nc) as tc, ExitStack() as top:
        consts = top.enter_context(tc.tile_pool(name="consts", bufs=1))
        gq_sb = consts.tile([128, KO], F32)
        bqgq_sb = consts.tile([128, KO], F32)
        gk_sb = consts.tile([128, MSH], F32)
        bkgk_sb = consts.tile([128, MSH], F32)
        ones_sb = consts.tile([128, 1], mm)
        ones4 = consts.tile([4, 1], mm)
        eps_sb = consts.tile([1, 1], F32)
        eps128_sb = consts.tile([1, 1], F32)
        nc.scalar.dma_start(gq_sb, gq_pm.ap())
        nc.scalar.dma_start(bqgq_sb, bqgq_pm.ap())
        nc.scalar.dma_start(gk_sb, gk_pm.ap())
        nc.scalar.dma_start(bkgk_sb, bkgk_pm.ap())
        nc.vector.memset(ones_sb, 1.0)
        nc.vector.memset(ones4, 1.0)
        nc.vector.memset(eps_sb, EPS)
        nc.vector.memset(eps128_sb, 128.0 * EPS)

        # k^T and v (full, gathered) live across attention; freed before Oproj
        with ExitStack() as acts_scope:
            act_pool = acts_scope.enter_context(tc.tile_pool(name="acts", bufs=1))
            kT_sb = act_pool.tile([128, KO, L], mm)
            v_sb = act_pool.tile([128, LSUB, D], mm)
            kinv_rep = act_pool.tile([128, L], F32, name="kinv_rep")
            ss4_sb = act_pool.tile([4, L], mm, name="ss4")

            # =========== K + V shard (this core's 10 heads) ===========
            with ExitStack() as ph:
                cpool = ph.enter_context(tc.tile_pool(name="ctx", bufs=1))
                wpool = ph.enter_context(tc.tile_pool(name="wkv", bufs=2))
                spool = ph.enter_context(tc.tile_pool(name="kscr", bufs=2))
                pp_mm = ph.enter_context(tc.tile_pool(name="ppkv", bufs=2,
                                                      space="PSUM"))
                pp_ss = ph.enter_context(tc.tile_pool(name="ppkss", bufs=1,
                                                      space="PSUM"))

                cT_sb = cpool.tile([128, KO, L], mm)
                cT_r = cT_p.rearrange("p (ko l) -> p ko l", ko=KO)
                # interleave first wk tiles with the cT quarters so the K
                # matmuls (per-ko granular) start as early as possible
                wdma(0, cT_sb[:, bass.ts(0, 10), :], cT_r[:, bass.ts(0, 10), :])
                wdma(1, cT_sb[:, bass.ts(1, 10), :], cT_r[:, bass.ts(1, 10), :])
                wk_tiles = []
                for t in range(2):
                    wk_sb = wpool.tile([128, KO, 256], mm, tag="w", name="wk_sb")
                    wdma2(wk_sb,
                          wk_p.ap()[t].rearrange("p (ko c) -> p ko c", ko=KO))
                    wk_tiles.append(wk_sb)
                wdma(0, cT_sb[:, bass.ts(2, 10), :], cT_r[:, bass.ts(2, 10), :])
                wdma(1, cT_sb[:, bass.ts(3, 10), :], cT_r[:, bass.ts(3, 10), :])
                bv_rep = cpool.tile([128, VSH], mm, name="bv_rep")
                nc.gpsimd.dma_start(bv_rep,
                                    bv_sh.ap()[None, :].to_broadcast([128, VSH]))
                kTs = cpool.tile([128, MSH, L], mm, name="kTs")

                ss_ps = pp_ss.tile([128, 512], F32, name="ps_kss")
                sq_prev = None
                for t in range(WKT):
                    if t < 2:
                        wk_sb = wk_tiles[t]
                    else:
                        wk_sb = wpool.tile([128, KO, 256], mm, tag="w",
                                           name="wk_sb")
                        wdma2(wk_sb,
                              wk_p.ap()[t].rearrange("p (ko c) -> p ko c",
                                                     ko=KO))
                    for mi in range(2):
                        m = 2 * t + mi
                        ps = pp_mm.tile([128, 512], F32, tag="mm", name="ps_k")
                        for ko in range(KO):
                            nc.tensor.matmul(ps, wk_sb[:, ko, bass.ts(mi, 128)],
                                             cT_sb[:, ko, :],
                                             start=(ko == 0), stop=(ko == KO - 1))
                        # k~ = gk*(Wk c + bk): fused scale+bias eviction
                        nc.scalar.activation(kTs[:, m, :], ps,
                                             mybir.ActivationFunctionType.Identity,
                                             bias=bkgk_sb[:, m:m + 1],
                                             scale=gk_sb[:, m:m + 1])
                        sq = spool.tile([128, 512], mm, tag="sq", name="sq")
                        nc.vector.tensor_mul(sq, kTs[:, m, :], kTs[:, m, :])
                        # delayed by one m so the evict->square chain never
                        # stalls the PE stream
                        if sq_prev is not None:
                            nc.tensor.matmul(ss_ps[:1, :L], ones_sb, sq_prev,
                                             start=(m == 1), stop=False)
                        sq_prev = sq
                nc.tensor.matmul(ss_ps[:1, :L], ones_sb, sq_prev,
                                 start=False, stop=True)
                ssk_bf = cpool.tile([1, L], mm, name="ssk_bf")
                nc.scalar.activation(ssk_bf, ss_ps[:1, :L],
                                     mybir.ActivationFunctionType.Copy)
                # spill K~^T shard + partial ss row into the AG input buffer
                nc.gpsimd.dma_start(
                    kv_in.ap()[:K_ELEMS].rearrange("(m p l) -> p m l",
                                                   m=MSH, p=128, l=L), kTs)
                nc.gpsimd.dma_start(
                    kv_in.ap()[K_ELEMS:K_ELEMS + SS_ELEMS][None, :], ssk_bf)

                # ---- V shard ----
                vs = cpool.tile([128, LSUB, VSH], mm, name="vs")
                for t in range(WKT):
                    wv_sb = wpool.tile([128, KO, 256], mm, tag="w", name="wv_sb")
                    wdma2(wv_sb,
                          wv_p.ap()[t].rearrange("p (ko c) -> p ko c", ko=KO))
                    for lb in range(LSUB):
                        ps = pp_mm.tile([128, 512], F32, tag="mm",
                                        name="ps_v")[:, :256]
                        for ko in range(KO):
                            nc.tensor.matmul(ps, cT_sb[:, ko, bass.ts(lb, 128)],
                                             wv_sb[:, ko, :],
                                             start=(ko == 0), stop=(ko == KO - 1))
                        nc.vector.tensor_add(vs[:, lb, bass.ts(t, 256)], ps,
                                             bv_rep[:, bass.ts(t, 256)])
                nc.gpsimd.dma_start(
                    kv_in.ap()[K_ELEMS + SS_ELEMS:].rearrange(
                        "(lb p n) -> p lb n", lb=LSUB, p=128, n=VSH), vs)

            # =========== AllGather K/V within each cfg group ===========
            nc.gpsimd.collective_compute(
                "AllGather", mybir.AluOpType.bypass,
                replica_groups=replica_groups,
                ins=[kv_in.ap()], outs=[kv_out.ap()])
            for r in range(R):
                base = r * SHARD_ELEMS
                nc.gpsimd.dma_start(
                    kT_sb[:, r * MSH:(r + 1) * MSH, :],
                    kv_out.ap()[base:base + K_ELEMS].rearrange(
                        "(m p l) -> p m l", m=MSH, p=128, l=L))
                nc.gpsimd.dma_start(
                    v_sb[:, :, r * VSH:(r + 1) * VSH],
                    kv_out.ap()[base + K_ELEMS + SS_ELEMS:base + SHARD_ELEMS]
                    .rearrange("(lb p n) -> p lb n", lb=LSUB, p=128, n=VSH))
            nc.gpsimd.dma_start(
                ss4_sb,
                kv_out.ap().rearrange("(r x) -> r x", r=R)[:, K_ELEMS:K_ELEMS +
                                                           SS_ELEMS])

            # ===== Q projection: one pass over Wq, q^T spilled to DRAM =====
            qsc_a = act_pool.tile([1, QCH], F32, name="qsc_a")
            qsc_b = act_pool.tile([1, QCH], F32, name="qsc_b")
            qsc_h = [qsc_a, qsc_b]
            with ExitStack() as qproj:
                hpool = qproj.enter_context(tc.tile_pool(name="hq", bufs=1))
                wpool = qproj.enter_context(tc.tile_pool(name="wq", bufs=2))
                qmpool = qproj.enter_context(tc.tile_pool(name="qtm", bufs=3))
                spool = qproj.enter_context(tc.tile_pool(name="qscr", bufs=2))
                pp_mm = qproj.enter_context(
                    tc.tile_pool(name="ppmmq", bufs=2, space="PSUM"))
                pp_ss = qproj.enter_context(
                    tc.tile_pool(name="ppqss", bufs=1, space="PSUM"))

                hT_sb = hpool.tile([128, KO, S_SHARD], mm)
                hT_r = hT_p.rearrange("p (ko s) -> p ko s", ko=KO)
                # emission order = queue order: first wq tiles slot between
                # the hT quarters on each queue, and all hT writes are
                # emitted before any matmul reads them
                wdma(0, hT_sb[:, bass.ts(0, 10), :], hT_r[:, bass.ts(0, 10), :])
                wdma(1, hT_sb[:, bass.ts(1, 10), :], hT_r[:, bass.ts(1, 10), :])
                wq_tiles = []
                for m in range(2):
                    wq_sb = wpool.tile([128, KO, 128], mm, tag="w",
                                       name="wq_sb")
                    wdma(m, wq_sb, wq_p.ap()[m].rearrange(
                        "p (ko c) -> p ko c", ko=KO))
                    wq_tiles.append(wq_sb)
                wdma(0, hT_sb[:, bass.ts(2, 10), :], hT_r[:, bass.ts(2, 10), :])
                wdma(1, hT_sb[:, bass.ts(3, 10), :], hT_r[:, bass.ts(3, 10), :])
                ss_a = pp_ss.tile([128, 512], F32, name="ps_qss_a")
                ss_b = pp_ss.tile([128, 512], F32, name="ps_qss_b")
                ss_h = [ss_a, ss_b]
                sq_prev = [None, None]
                for m in range(KO):
                    if m < 2:
                        wq_sb = wq_tiles[m]
                    else:
                        wq_sb = wpool.tile([128, KO, 128], mm, tag="w",
                                           name="wq_sb")
                        wdma(m, wq_sb, wq_p.ap()[m].rearrange(
                            "p (ko c) -> p ko c", ko=KO))
                    qTm = qmpool.tile([128, 2, QCH], mm, tag="qtm", name="qTm")
                    for hf in range(2):
                        ps = pp_mm.tile([128, 512], F32, tag="mm", name="ps_q")
                        for ko in range(KO):
                            nc.tensor.matmul(
                                ps, wq_sb[:, ko, :],
                                hT_sb[:, ko, bass.ts(hf, QCH)],
                                start=(ko == 0), stop=(ko == KO - 1))
                        nc.scalar.activation(
                            qTm[:, hf, :], ps,
                            mybir.ActivationFunctionType.Identity,
                            bias=bqgq_sb[:, m:m + 1], scale=gq_sb[:, m:m + 1])
                        sq = spool.tile([128, 512], mm, tag=f"sq{hf}",
                                        name="sq")
                        nc.vector.tensor_mul(sq, qTm[:, hf, :], qTm[:, hf, :])
                        if sq_prev[hf] is not None:
                            nc.tensor.matmul(ss_h[hf][:1, :QCH], ones_sb,
                                             sq_prev[hf],
                                             start=(m == 1), stop=False)
                        sq_prev[hf] = sq
                    wdma(m, qT_dram.ap()[m], qTm)
                    if m == 25:
                        # kinv from the AG'd exact partial ss rows — mid-Q
                        # so the collective is long done and the tiny PE op
                        # never stalls the stream
                        ps4 = pp_mm.tile([128, 512], F32, tag="mm",
                                         name="ps4")[:1, :L]
                        nc.tensor.matmul(ps4, ones4, ss4_sb,
                                         start=True, stop=True)
                        kroot = act_pool.tile([1, L], F32, name="kroot")
                        nc.scalar.activation(
                            kroot, ps4, mybir.ActivationFunctionType.Sqrt,
                            scale=1.0 / D, bias=eps_sb)
                        kinv = act_pool.tile([1, L], F32, name="kinv")
                        nc.vector.reciprocal_approx_fast(kinv, kroot)
                        nc.gpsimd.partition_broadcast(kinv_rep, kinv)
                        for g in range(KO // 8):
                            nc.vector.tensor_mul(
                                kT_sb[:, bass.ts(g, 8), :],
                                kT_sb[:, bass.ts(g, 8), :],
                                kinv_rep[:, None, :].to_broadcast([128, 8, L]))
                for hf in range(2):
                    nc.tensor.matmul(ss_h[hf][:1, :QCH], ones_sb, sq_prev[hf],
                                     start=False, stop=True)
                    # qsc = scale / rms(q) per s column (scale folded into
                    # the sqrt)
                    qroot = spool.tile([1, QCH], F32, name="qroot", tag="qsc")
                    nc.scalar.activation(qroot, ss_h[hf][:1, :QCH],
                                         mybir.ActivationFunctionType.Sqrt,
                                         scale=128.0 / D, bias=eps128_sb)
                    nc.vector.reciprocal_approx_fast(qsc_h[hf], qroot)

            # ========== attention: logits transposed [L-part, s-free] ======
            with ExitStack() as at_scope:
                qcpool = at_scope.enter_context(tc.tile_pool(name="qtc",
                                                             bufs=2))
                rpool = at_scope.enter_context(tc.tile_pool(name="qrep",
                                                            bufs=2))
                spool = at_scope.enter_context(tc.tile_pool(name="ascr",
                                                            bufs=2))
                apool = at_scope.enter_context(tc.tile_pool(name="attn",
                                                            bufs=2))
                opool = at_scope.enter_context(tc.tile_pool(name="oev",
                                                            bufs=2))
                pp_pt = at_scope.enter_context(
                    tc.tile_pool(name="pppt", bufs=2, space="PSUM"))
                pp_sr = at_scope.enter_context(
                    tc.tile_pool(name="ppsr", bufs=2, space="PSUM"))
                pp_o = at_scope.enter_context(
                    tc.tile_pool(name="ppo", bufs=2, space="PSUM"))
                qT_rd = qT_dram.rearrange("m p s -> p m s")
                for s0 in range(NSUB):
                    csl = bass.ts(s0, SCH)
                    qTc = qcpool.tile([128, KO, SCH], mm, tag="qtc",
                                      name="qTc")
                    nc.sync.dma_start(qTc[:, :20, :], qT_rd[:, :20, csl])
                    nc.scalar.dma_start(qTc[:, 20:, :], qT_rd[:, 20:, csl])
                    qsc_rep = rpool.tile([128, SCH], F32, tag="qr",
                                         name="qsc_rep")
                    nc.gpsimd.partition_broadcast(
                        qsc_rep, qsc_h[s0 // 2][:, bass.ts(s0 % 2, SCH)])
                    for g in range(KO // 8):
                        nc.vector.tensor_mul(
                            qTc[:, bass.ts(g, 8), :], qTc[:, bass.ts(g, 8), :],
                            qsc_rep[:, None, :].to_broadcast([128, 8, SCH]))
                    for h in range(H):
                        pt = pp_pt.tile([128, LSUB, SCH], F32, tag="pt",
                                        name="pt")
                        for lb in range(LSUB):
                            nc.tensor.matmul(
                                pt[:, lb, :], kT_sb[:, h, bass.ts(lb, 128)],
                                qTc[:, h, :],
                                start=(lb % 2 == 0), stop=(lb % 2 == 1))
                        probsT = apool.tile([128, LSUB, SCH], mm,
                                            tag="probsT")
                        nc.scalar.activation(probsT, pt,
                                             mybir.ActivationFunctionType.Exp)
                        sr = pp_sr.tile([128, 512], F32, tag="sr", name="sr")
                        for lb in range(LSUB):
                            nc.tensor.matmul(sr[:1, :SCH], ones_sb,
                                             probsT[:, lb, :],
                                             start=(lb == 0),
                                             stop=(lb == LSUB - 1))
                        rinv = spool.tile([1, SCH], F32, tag="rinv",
                                          name="rinv")
                        nc.vector.reciprocal_approx_fast(rinv, sr[:1, :SCH])
                        ops = pp_o.tile([128, SCH], F32, tag="o", name="ops")
                        for lb in range(LSUB):
                            nc.tensor.matmul(ops, v_sb[:, lb, bass.ts(h, 128)],
                                             probsT[:, lb, :],
                                             start=(lb == 0),
                                             stop=(lb == LSUB - 1))
                        # replicate 1/sum across partitions off the PE
                        # stream (gpsimd is otherwise idle here)
                        rrep = spool.tile([128, SCH], F32, tag="rrep",
                                          name="rrep")
                        nc.gpsimd.partition_broadcast(rrep, rinv)
                        o_h = opool.tile([128, SCH], mm, tag="oh", name="o_h")
                        nc.vector.tensor_mul(o_h, ops, rrep)
                        nc.sync.dma_start(oT_r[:, h, bass.ts(s0, SCH)], o_h)

        # =========== output projection ===========
        with ExitStack() as ph:
            opool = ph.enter_context(tc.tile_pool(name="oT", bufs=1))
            wpool = ph.enter_context(tc.tile_pool(name="wo", bufs=2))
            spool = ph.enter_context(tc.tile_pool(name="oscr", bufs=3))
            pp_mm = ph.enter_context(tc.tile_pool(name="ppmmo", bufs=2,
                                                  space="PSUM"))

            oT_all = opool.tile([128, KO, S_SHARD], mm)
            bo_rep = opool.tile([128, D], mm, name="bo_rep")
            nc.gpsimd.dma_start(bo_rep, bot.ap()[None, :].to_broadcast([128, D]))
            # the scalar queue drains its attention work early, so the first
            # three oT readbacks transfer during the attention tail; the
            # spill-gated last readback and the first wo tile race on both
            # queues; wo t1 rides the SWDGE
            for c in range(NSUB - 1):
                nc.scalar.dma_start(oT_all[:, :, bass.ts(c, SCH)],
                                    oT_r[:, :, bass.ts(c, SCH)])
            csl = bass.ts(NSUB - 1, SCH)
            nc.sync.dma_start(oT_all[:, :20, csl], oT_r[:, :20, csl])
            nc.scalar.dma_start(oT_all[:, 20:, csl], oT_r[:, 20:, csl])
            wo_tiles = []
            for t in range(2):
                wo_sb = wpool.tile([128, KO, 512], mm, tag="wo", name="wo_sb")
                wo_src = wo_p.ap()[t].rearrange("p (ko c) -> p ko c", ko=KO)
                if t == 0:
                    wdma2(wo_sb, wo_src)
                else:
                    nc.gpsimd.dma_start(wo_sb, wo_src)
                wo_tiles.append(wo_sb)
            for t in range(D // 512):
                if t < 2:
                    wo_sb = wo_tiles[t]
                else:
                    wo_sb = wpool.tile([128, KO, 512], mm, tag="wo",
                                       name="wo_sb")
                    wdma(t, wo_sb,
                         wo_p.ap()[t].rearrange("p (ko c) -> p ko c", ko=KO))
                for cs in range(S_SHARD // 128):
                    ps = pp_mm.tile([128, 512], F32, tag="mm", name="ps_o")
                    for ko in range(KO):
                        nc.tensor.matmul(ps, oT_all[:, ko, bass.ts(cs, 128)],
                                         wo_sb[:, ko, :],
                                         start=(ko == 0), stop=(ko == KO - 1))
                    o_sb = spool.tile([128, 512], mm, tag="out", name="o_sb")
                    nc.vector.tensor_add(o_sb, ps, bo_rep[:, bass.ts(t, 512)])
                    nc.scalar.dma_start(out_r[:, cs, bass.ts(t, 512)], o_sb)

    nc.compile()
    return nc


def _get_nc():
    global _CACHED_NC
    if _CACHED_NC is None:
        _CACHED_NC = _build()
    return _CACHED_NC


def _pack_w(wT, tc):
    """[D, N] (contraction-major transposed weight) -> [N//tc, 128, KO*tc]
    so each streamed tile is one fully-contiguous DMA read."""
    n = wT.shape[1]
    nt = n // tc
    return np.ascontiguousarray(
        wT.reshape(KO, 128, nt, tc).transpose(2, 1, 0, 3).reshape(
            nt, 128, KO * tc))


def kernel(hidden_cond, hidden_uncond, context_cond, context_uncond,
           Wq, bq, Wkv, bkv, gq, gk, Wo, bo):
    global LAST_EXEC_NS
    import ml_dtypes
    bf = ml_dtypes.bfloat16 if MM == mybir.dt.bfloat16 else np.float32
    f32 = np.float32

    nc = _get_nc()

    hid = [np.asarray(hidden_cond, f32).reshape(-1, D),
           np.asarray(hidden_uncond, f32).reshape(-1, D)]
    ctxs = [np.asarray(context_cond, f32).reshape(-1, D),
            np.asarray(context_uncond, f32).reshape(-1, D)]
    Wq = np.asarray(Wq, f32)
    Wkv = np.asarray(Wkv, f32)
    Wo = np.asarray(Wo, f32)
    bq = np.asarray(bq, f32)
    bkv = np.asarray(bkv, f32)
    bo = np.asarray(bo, f32)
    gq = np.asarray(gq, f32)
    gk = np.asarray(gk, f32)
    bk, bv = bkv[:D], bkv[D:]

    wq_pk = _pack_w(np.ascontiguousarray(Wq.T).astype(bf), 128)
    wo_pk = _pack_w(np.ascontiguousarray(Wo.T).astype(bf), 512)
    WkT = np.ascontiguousarray(Wkv[:D].T).astype(bf)
    WvT = np.ascontiguousarray(Wkv[D:].T).astype(bf)
    wk_pks = [_pack_w(WkT[:, r * VSH:(r + 1) * VSH], 256) for r in range(R)]
    wv_pks = [_pack_w(WvT[:, r * VSH:(r + 1) * VSH], 256) for r in range(R)]

    common = {
        "wq_p": wq_pk, "wo_p": wo_pk,
        "gq_pm": np.ascontiguousarray(gq.reshape(KO, 128).T),
        "bqgq_pm": np.ascontiguousarray((bq * gq).reshape(KO, 128).T),
        "bo": bo,
    }
    cT_ps = []
    for g in range(2):
        cT = np.ascontiguousarray(ctxs[g].T).astype(bf)   # [D, L]
        cT_ps.append(np.ascontiguousarray(
            cT.reshape(KO, 128, L).transpose(1, 0, 2).reshape(128, KO * L)))

    in_maps = []
    for core in range(8):
        g, r = core // 4, core % 4
        hT = np.ascontiguousarray(
            hid[g][r * S_SHARD:(r + 1) * S_SHARD].T).astype(bf)  # [D, S_SHARD]
        hT_pk = np.ascontiguousarray(
            hT.reshape(KO, 128, S_SHARD).transpose(1, 0, 2)
            .reshape(128, KO * S_SHARD))
        sl = slice(r * VSH, (r + 1) * VSH)
        in_maps.append({
            "hT_p": hT_pk, "cT_p": cT_ps[g],
            "wk_p": wk_pks[r], "wv_p": wv_pks[r],
            "gk_pm": np.ascontiguousarray(gk[sl].reshape(MSH, 128).T),
            "bkgk_pm": np.ascontiguousarray((bk * gk)[sl].reshape(MSH, 128).T),
            "bv_sh": np.ascontiguousarray(bv[sl]),
            **common,
        })

    res = bass_utils.run_bass_kernel_spmd(nc, in_maps, list(range(8)),
                                          trace=TRACE)
    LAST_EXEC_NS = res.exec_time_ns

    out_c = np.concatenate(
        [np.asarray(res.results[i]["out"], f32) for i in range(4)], axis=0)
    out_u = np.concatenate(
        [np.asarray(res.results[i]["out"], f32) for i in range(4, 8)], axis=0)
    return (out_c[None], out_u[None])


# revision 44
# speedup vs baseline: 1.0255x; 1.0255x over previous
"""CFG dual cross-attention on 8 Trainium2 NeuronCores (Bass/Tile).

Sharding: the cfg axis (cond/uncond) splits the 8 cores into 2 groups of 4;
within a group the 4096 query rows are sharded 4-way (1024 rows/core) and the
K/V projection is sharded 4-way over heads.  Each core computes K^T/V for its
10 heads, the group AllGathers K/V (plus exact partial sum-of-squares rows for
the K rms-norm), and every core then runs all 40 heads of attention over its
own query rows.  The host concatenates the row shards.

Matmul operands are bf16 (fp32 PSUM accumulation); softmax/rms statistics in
fp32.  All weights are repacked host-side so every streamed weight tile is a
single fully-contiguous DMA read, and hT/cT are packed per-chunk contiguous.

Attention uses the transposed-logits formulation: logits [L-part, s-free] per
head, exp on the scalar engine, key-axis sum via ones-matmul, softmax 1/sum
via the fast approx reciprocal, replicated across partitions with a rank-1
f32r matmul issued *after* the A@V matmuls so the reciprocal latency hides
under PE work.  Per-m rms sum-of-squares matmuls are delayed by one m-tile so
the eviction->square chain never stalls the PE stream.
"""

from contextlib import ExitStack

import numpy as np

import concourse.bass as bass
import concourse.bacc as bacc
import concourse.mybir as mybir
import concourse.tile as tile
from concourse import bass_utils

EPS = 1e-6
F32 = mybir.dt.float32
F32R = mybir.dt.float32r

# ---- problem shape (nn_CFGDualCrossAttention: D=5120, H=40, S=4096, L=512) ----
D = 5120
L = 512
S_SHARD = 1024        # 4096 / 4 cores per cfg group
KO = D // 128         # contraction subtiles == heads (head_dim 128)
H = KO
LSUB = L // 128
QCH = 512             # q projection chunk (2 per shard)
SCH = 256             # attention sub-chunk (2 per q chunk)
NSUB = S_SHARD // SCH
R = 4                 # cores per cfg group
MSH = KO // R         # kv-shard m-tiles (10)
VSH = D // R          # kv-shard output cols (1280)
MM = mybir.dt.bfloat16
SCALE = float(128 ** -0.5)

# AllGather buffer layout (bf16 elements)
K_ELEMS = MSH * 128 * L           # 655360
SS_ELEMS = L                      # 512  (partial sum-of-squares row)
V_ELEMS = LSUB * 128 * VSH        # 655360
SHARD_ELEMS = K_ELEMS + SS_ELEMS + V_ELEMS

TRACE = False         # set by test harness for NTFF timing
LAST_EXEC_NS = None
_CACHED_NC = None


def _build() -> bacc.Bacc:
    mm = MM
    WKT = 5           # wk/wv stream tiles (256 cols each)
    WOT = D // 256    # wo stream tiles

    nc = bacc.Bacc("TRN2", target_bir_lowering=False, debug=False, num_devices=8)

    # ---- external inputs (host-side repacked; see kernel() below) ----
    hT_p = nc.dram_tensor("hT_p", [128, KO * S_SHARD], mm,
                          kind="ExternalInput")
    cT_p = nc.dram_tensor("cT_p", [128, KO * L], mm, kind="ExternalInput")
    wq_p = nc.dram_tensor("wq_p", [KO, 128, KO * 128], mm, kind="ExternalInput")
    wk_p = nc.dram_tensor("wk_p", [WKT, 128, KO * 256], mm, kind="ExternalInput")
    wv_p = nc.dram_tensor("wv_p", [WKT, 128, KO * 256], mm, kind="ExternalInput")
    wo_p = nc.dram_tensor("wo_p", [WOT, 128, KO * 256], mm, kind="ExternalInput")
    gq_pm = nc.dram_tensor("gq_pm", [128, KO], F32, kind="ExternalInput")
    bqgq_pm = nc.dram_tensor("bqgq_pm", [128, KO], F32, kind="ExternalInput")
    gk_pm = nc.dram_tensor("gk_pm", [128, MSH], F32, kind="ExternalInput")
    bkgk_pm = nc.dram_tensor("bkgk_pm", [128, MSH], F32, kind="ExternalInput")
    bv_sh = nc.dram_tensor("bv_sh", [VSH], F32, kind="ExternalInput")
    bot = nc.dram_tensor("bo", [D], F32, kind="ExternalInput")
    out = nc.dram_tensor("out", [S_SHARD, D], mm, kind="ExternalOutput")

    oT_dram = nc.dram_tensor("oT_spill", [D, S_SHARD], mm)
    qT_dram = nc.dram_tensor("qT_spill", [KO, 128, S_SHARD], mm)
    kv_in = nc.dram_tensor("kv_in", [SHARD_ELEMS], mm)
    # note: Shared addr_space needs >4-core groups; Local costs one extra copy
    kv_out = nc.dram_tensor("kv_out", [R * SHARD_ELEMS], mm)

    oT_r = oT_dram.rearrange("(ko p) s -> p ko s", p=128)
    out_r = out.rearrange("(cs p) n -> p cs n", p=128)

    replica_groups = [[0, 1, 2, 3], [4, 5, 6, 7]]

    def wdma(i, dst, src):
        # alternate big streaming DMAs across the two HWDGE queues
        (nc.sync if i % 2 == 0 else nc.scalar).dma_start(dst, src)

    def wdma2(dst, src):
        # split one weight tile across both HWDGE queues (halved latency)
        half = dst.shape[1] // 2
        nc.sync.dma_start(dst[:, :half], src[:, :half])
        nc.scalar.dma_start(dst[:, half:], src[:, half:])

    with tile.TileContext(nc) as tc, ExitStack() as top:
        consts = top.enter_context(tc.tile_pool(name="consts", bufs=1))
        gq_sb = consts.tile([128, KO], F32)
        bqgq_sb = consts.tile([128, KO], F32)
        gk_sb = consts.tile([128, MSH], F32)
        bkgk_sb = consts.tile([128, MSH], F32)
        ones_sb = consts.tile([128, 1], mm)
        ones4 = consts.tile([4, 1], mm)
        eps_sb = consts.tile([1, 1], F32)
        eps128_sb = consts.tile([1, 1], F32)
        nc.scalar.dma_start(gq_sb, gq_pm.ap())
        nc.scalar.dma_start(bqgq_sb, bqgq_pm.ap())
        nc.scalar.dma_start(gk_sb, gk_pm.ap())
        nc.scalar.dma_start(bkgk_sb, bkgk_pm.ap())
        nc.vector.memset(ones_sb, 1.0)
        nc.vector.memset(ones4, 1.0)
        nc.vector.memset(eps_sb, EPS)
        nc.vector.memset(eps128_sb, 128.0 * EPS)

        # k^T and v (full, gathered) live across attention; freed before Oproj
        with ExitStack() as acts_scope:
            act_pool = acts_scope.enter_context(tc.tile_pool(name="acts", bufs=1))
            kT_sb = act_pool.tile([128, KO, L], mm)
            v_sb = act_pool.tile([128, LSUB, D], mm)
            kinv_rep = act_pool.tile([128, L], F32, name="kinv_rep")
            ss4_sb = act_pool.tile([4, L], mm, name="ss4")

            # =========== K + V shard (this core's 10 heads) ===========
            with ExitStack() as ph:
                cpool = ph.enter_context(tc.tile_pool(name="ctx", bufs=1))
                wpool = ph.enter_context(tc.tile_pool(name="wkv", bufs=2))
                spool = ph.enter_context(tc.tile_pool(name="kscr", bufs=2))
                pp_mm = ph.enter_context(tc.tile_pool(name="ppkv", bufs=2,
                                                      space="PSUM"))
                pp_ss = ph.enter_context(tc.tile_pool(name="ppkss", bufs=1,
                                                      space="PSUM"))

                cT_sb = cpool.tile([128, KO, L], mm)
                cT_r = cT_p.rearrange("p (ko l) -> p ko l", ko=KO)
                # interleave first wk tiles with the cT quarters so the K
                # matmuls (per-ko granular) start as early as possible
                wdma(0, cT_sb[:, bass.ts(0, 10), :], cT_r[:, bass.ts(0, 10), :])
                wdma(1, cT_sb[:, bass.ts(1, 10), :], cT_r[:, bass.ts(1, 10), :])
                wk_tiles = []
                for t in range(2):
                    wk_sb = wpool.tile([128, KO, 256], mm, tag="w", name="wk_sb")
                    wdma2(wk_sb,
                          wk_p.ap()[t].rearrange("p (ko c) -> p ko c", ko=KO))
                    wk_tiles.append(wk_sb)
                wdma(0, cT_sb[:, bass.ts(2, 10), :], cT_r[:, bass.ts(2, 10), :])
                wdma(1, cT_sb[:, bass.ts(3, 10), :], cT_r[:, bass.ts(3, 10), :])
                bv_rep = cpool.tile([128, VSH], mm, name="bv_rep")
                nc.gpsimd.dma_start(bv_rep,
                                    bv_sh.ap()[None, :].to_broadcast([128, VSH]))
                kTs = cpool.tile([128, MSH, L], mm, name="kTs")

                ss_ps = pp_ss.tile([128, 512], F32, name="ps_kss")
                sq_prev = None
                for t in range(WKT):
                    if t < 2:
                        wk_sb = wk_tiles[t]
                    else:
                        wk_sb = wpool.tile([128, KO, 256], mm, tag="w",
                                           name="wk_sb")
                        wdma2(wk_sb,
                              wk_p.ap()[t].rearrange("p (ko c) -> p ko c",
                                                     ko=KO))
                    for mi in range(2):
                        m = 2 * t + mi
                        ps = pp_mm.tile([128, 512], F32, tag="mm", name="ps_k")
                        for ko in range(KO):
                            nc.tensor.matmul(ps, wk_sb[:, ko, bass.ts(mi, 128)],
                                             cT_sb[:, ko, :],
                                             start=(ko == 0), stop=(ko == KO - 1))
                        # k~ = gk*(Wk c + bk): fused scale+bias eviction
                        nc.scalar.activation(kTs[:, m, :], ps,
                                             mybir.ActivationFunctionType.Identity,
                                             bias=bkgk_sb[:, m:m + 1],
                                             scale=gk_sb[:, m:m + 1])
                        sq = spool.tile([128, 512], mm, tag="sq", name="sq")
                        nc.vector.tensor_mul(sq, kTs[:, m, :], kTs[:, m, :])
                        # delayed by one m so the evict->square chain never
                        # stalls the PE stream
                        if sq_prev is not None:
                            nc.tensor.matmul(ss_ps[:1, :L], ones_sb, sq_prev,
                                             start=(m == 1), stop=False)
                        sq_prev = sq
                nc.tensor.matmul(ss_ps[:1, :L], ones_sb, sq_prev,
                                 start=False, stop=True)
                ssk_bf = cpool.tile([1, L], mm, name="ssk_bf")
                nc.scalar.activation(ssk_bf, ss_ps[:1, :L],
                                     mybir.ActivationFunctionType.Copy)
                # spill K~^T shard + partial ss row into the AG input buffer
                nc.scalar.dma_start(
                    kv_in.ap()[:K_ELEMS].rearrange("(m p l) -> p m l",
                                                   m=MSH, p=128, l=L), kTs)
                nc.scalar.dma_start(
                    kv_in.ap()[K_ELEMS:K_ELEMS + SS_ELEMS][None, :], ssk_bf)

                # ---- V shard ----
                vs = cpool.tile([128, LSUB, VSH], mm, name="vs")
                for t in range(WKT):
                    wv_sb = wpool.tile([128, KO, 256], mm, tag="w", name="wv_sb")
                    wdma2(wv_sb,
                          wv_p.ap()[t].rearrange("p (ko c) -> p ko c", ko=KO))
                    for lb in range(LSUB):
                        ps = pp_mm.tile([128, 512], F32, tag="mm",
                                        name="ps_v")[:, :256]
                        for ko in range(KO):
                            nc.tensor.matmul(ps, cT_sb[:, ko, bass.ts(lb, 128)],
                                             wv_sb[:, ko, :],
                                             start=(ko == 0), stop=(ko == KO - 1))
                        nc.vector.tensor_add(vs[:, lb, bass.ts(t, 256)], ps,
                                             bv_rep[:, bass.ts(t, 256)])
                # split across both queues: this spill completes only at the
                # V-proj tail, and whichever queue carries it stalls its
                # share of the Q-phase prefetch behind it
                vdst = kv_in.ap()[K_ELEMS + SS_ELEMS:].rearrange(
                    "(lb p n) -> p lb n", lb=LSUB, p=128, n=VSH)
                nc.sync.dma_start(vdst[:, :2, :], vs[:, :2, :])
                nc.scalar.dma_start(vdst[:, 2:, :], vs[:, 2:, :])

            # =========== AllGather K/V within each cfg group ===========
            nc.gpsimd.collective_compute(
                "AllGather", mybir.AluOpType.bypass,
                replica_groups=replica_groups,
                ins=[kv_in.ap()], outs=[kv_out.ap()])
            for r in range(R):
                base = r * SHARD_ELEMS
                nc.gpsimd.dma_start(
                    kT_sb[:, r * MSH:(r + 1) * MSH, :],
                    kv_out.ap()[base:base + K_ELEMS].rearrange(
                        "(m p l) -> p m l", m=MSH, p=128, l=L))
                nc.gpsimd.dma_start(
                    v_sb[:, :, r * VSH:(r + 1) * VSH],
                    kv_out.ap()[base + K_ELEMS + SS_ELEMS:base + SHARD_ELEMS]
                    .rearrange("(lb p n) -> p lb n", lb=LSUB, p=128, n=VSH))
            nc.gpsimd.dma_start(
                ss4_sb,
                kv_out.ap().rearrange("(r x) -> r x", r=R)[:, K_ELEMS:K_ELEMS +
                                                           SS_ELEMS])

            # ===== Q projection: one pass over Wq, q^T spilled to DRAM =====
            qsc_a = act_pool.tile([1, QCH], F32, name="qsc_a")
            qsc_b = act_pool.tile([1, QCH], F32, name="qsc_b")
            qsc_h = [qsc_a, qsc_b]
            with ExitStack() as qproj:
                hpool = qproj.enter_context(tc.tile_pool(name="hq", bufs=1))
                wpool = qproj.enter_context(tc.tile_pool(name="wq", bufs=2))
                qmpool = qproj.enter_context(tc.tile_pool(name="qtm", bufs=3))
                spool = qproj.enter_context(tc.tile_pool(name="qscr", bufs=2))
                pp_mm = qproj.enter_context(
                    tc.tile_pool(name="ppmmq", bufs=2, space="PSUM"))
                pp_ss = qproj.enter_context(
                    tc.tile_pool(name="ppqss", bufs=1, space="PSUM"))

                hT_sb = hpool.tile([128, KO, S_SHARD], mm)
                hT_r = hT_p.rearrange("p (ko s) -> p ko s", ko=KO)
                # emission order = queue order: first wq tiles slot between
                # the hT quarters on each queue, and all hT writes are
                # emitted before any matmul reads them
                wdma(0, hT_sb[:, bass.ts(0, 10), :], hT_r[:, bass.ts(0, 10), :])
                wdma(1, hT_sb[:, bass.ts(1, 10), :], hT_r[:, bass.ts(1, 10), :])
                wq_tiles = []
                for m in range(2):
                    wq_sb = wpool.tile([128, KO, 128], mm, tag="w",
                                       name="wq_sb")
                    wdma(m, wq_sb, wq_p.ap()[m].rearrange(
                        "p (ko c) -> p ko c", ko=KO))
                    wq_tiles.append(wq_sb)
                wdma(0, hT_sb[:, bass.ts(2, 10), :], hT_r[:, bass.ts(2, 10), :])
                wdma(1, hT_sb[:, bass.ts(3, 10), :], hT_r[:, bass.ts(3, 10), :])
                ss_a = pp_ss.tile([128, 512], F32, name="ps_qss_a")
                ss_b = pp_ss.tile([128, 512], F32, name="ps_qss_b")
                ss_h = [ss_a, ss_b]
                sq_prev = [None, None]
                for m in range(KO):
                    if m < 2:
                        wq_sb = wq_tiles[m]
                    else:
                        wq_sb = wpool.tile([128, KO, 128], mm, tag="w",
                                           name="wq_sb")
                        wdma(m, wq_sb, wq_p.ap()[m].rearrange(
                            "p (ko c) -> p ko c", ko=KO))
                    qTm = qmpool.tile([128, 2, QCH], mm, tag="qtm", name="qTm")
                    for hf in range(2):
                        ps = pp_mm.tile([128, 512], F32, tag="mm", name="ps_q")
                        for ko in range(KO):
                            nc.tensor.matmul(
                                ps, wq_sb[:, ko, :],
                                hT_sb[:, ko, bass.ts(hf, QCH)],
                                start=(ko == 0), stop=(ko == KO - 1))
                        nc.scalar.activation(
                            qTm[:, hf, :], ps,
                            mybir.ActivationFunctionType.Identity,
                            bias=bqgq_sb[:, m:m + 1], scale=gq_sb[:, m:m + 1])
                        sq = spool.tile([128, 512], mm, tag=f"sq{hf}",
                                        name="sq")
                        nc.vector.tensor_mul(sq, qTm[:, hf, :], qTm[:, hf, :])
                        if sq_prev[hf] is not None:
                            nc.tensor.matmul(ss_h[hf][:1, :QCH], ones_sb,
                                             sq_prev[hf],
                                             start=(m == 1), stop=False)
                        sq_prev[hf] = sq
                    wdma(m, qT_dram.ap()[m], qTm)
                    if m == 25:
                        # kinv from the AG'd exact partial ss rows — mid-Q
                        # so the collective is long done and the tiny PE op
                        # never stalls the stream
                        ps4 = pp_mm.tile([128, 512], F32, tag="mm",
                                         name="ps4")[:1, :L]
                        nc.tensor.matmul(ps4, ones4, ss4_sb,
                                         start=True, stop=True)
                        kroot = act_pool.tile([1, L], F32, name="kroot")
                        nc.scalar.activation(
                            kroot, ps4, mybir.ActivationFunctionType.Sqrt,
                            scale=1.0 / D, bias=eps_sb)
                        kinv = act_pool.tile([1, L], F32, name="kinv")
                        nc.vector.reciprocal_approx_fast(kinv, kroot)
                        nc.gpsimd.partition_broadcast(kinv_rep, kinv)
                        for g in range(KO // 8):
                            nc.vector.tensor_mul(
                                kT_sb[:, bass.ts(g, 8), :],
                                kT_sb[:, bass.ts(g, 8), :],
                                kinv_rep[:, None, :].to_broadcast([128, 8, L]))
                for hf in range(2):
                    nc.tensor.matmul(ss_h[hf][:1, :QCH], ones_sb, sq_prev[hf],
                                     start=False, stop=True)
                    # qsc = scale / rms(q) per s column (scale folded into
                    # the sqrt)
                    qroot = spool.tile([1, QCH], F32, name="qroot", tag="qsc")
                    nc.scalar.activation(qroot, ss_h[hf][:1, :QCH],
                                         mybir.ActivationFunctionType.Sqrt,
                                         scale=128.0 / D, bias=eps128_sb)
                    nc.vector.reciprocal_approx_fast(qsc_h[hf], qroot)

            # ========== attention: logits transposed [L-part, s-free] ======
            with ExitStack() as at_scope:
                qcpool = at_scope.enter_context(tc.tile_pool(name="qtc",
                                                             bufs=2))
                rpool = at_scope.enter_context(tc.tile_pool(name="qrep",
                                                            bufs=2))
                spool = at_scope.enter_context(tc.tile_pool(name="ascr",
                                                            bufs=2))
                apool = at_scope.enter_context(tc.tile_pool(name="attn",
                                                            bufs=2))
                opool = at_scope.enter_context(tc.tile_pool(name="oev",
                                                            bufs=2))
                pp_pt = at_scope.enter_context(
                    tc.tile_pool(name="pppt", bufs=2, space="PSUM"))
                pp_sr = at_scope.enter_context(
                    tc.tile_pool(name="ppsr", bufs=2, space="PSUM"))
                pp_o = at_scope.enter_context(
                    tc.tile_pool(name="ppo", bufs=2, space="PSUM"))
                qT_rd = qT_dram.rearrange("m p s -> p m s")

                # sub-chunk prep is split so the next chunk's q^T readback
                # is issued mid-way through the current head loop (every
                # engine queue is lockstepped to the head cadence, so
                # anything emitted after the loop only starts at its end)
                def prep_load(s0):
                    csl = bass.ts(s0, SCH)
                    qTc = qcpool.tile([128, KO, SCH], mm, tag="qtc",
                                      name="qTc")
                    nc.sync.dma_start(qTc[:, :20, :], qT_rd[:, :20, csl])
                    nc.scalar.dma_start(qTc[:, 20:, :], qT_rd[:, 20:, csl])
                    qsc_rep = rpool.tile([128, SCH], F32, tag="qr",
                                         name="qsc_rep")
                    nc.gpsimd.partition_broadcast(
                        qsc_rep, qsc_h[s0 // 2][:, bass.ts(s0 % 2, SCH)])
                    return qTc, qsc_rep

                def prep_scale(qTc, qsc_rep):
                    for g in range(KO // 8):
                        nc.vector.tensor_mul(
                            qTc[:, bass.ts(g, 8), :], qTc[:, bass.ts(g, 8), :],
                            qsc_rep[:, None, :].to_broadcast([128, 8, SCH]))

                nxt = prep_load(0)
                prep_scale(*nxt)
                for s0 in range(NSUB):
                    qTc, _ = nxt
                    for h in range(H):
                        if s0 + 1 < NSUB and h == 8:
                            nxt = prep_load(s0 + 1)
                        if s0 + 1 < NSUB and h == 20:
                            # readback has landed by now; DVE never waits
                            prep_scale(*nxt)
                        pt = pp_pt.tile([128, LSUB, SCH], F32, tag="pt",
                                        name="pt")
                        for lb in range(LSUB):
                            nc.tensor.matmul(
                                pt[:, lb, :], kT_sb[:, h, bass.ts(lb, 128)],
                                qTc[:, h, :],
                                start=(lb % 2 == 0), stop=(lb % 2 == 1))
                        probsT = apool.tile([128, LSUB, SCH], mm,
                                            tag="probsT")
                        nc.scalar.activation(probsT, pt,
                                             mybir.ActivationFunctionType.Exp)
                        sr = pp_sr.tile([128, 512], F32, tag="sr", name="sr")
                        for lb in range(LSUB):
                            nc.tensor.matmul(sr[:1, :SCH], ones_sb,
                                             probsT[:, lb, :],
                                             start=(lb == 0),
                                             stop=(lb == LSUB - 1))
                        rinv = spool.tile([1, SCH], F32, tag="rinv",
                                          name="rinv")
                        nc.vector.reciprocal_approx_fast(rinv, sr[:1, :SCH])
                        ops = pp_o.tile([128, SCH], F32, tag="o", name="ops")
                        for lb in range(LSUB):
                            nc.tensor.matmul(ops, v_sb[:, lb, bass.ts(h, 128)],
                                             probsT[:, lb, :],
                                             start=(lb == 0),
                                             stop=(lb == LSUB - 1))
                        # replicate 1/sum across partitions off the PE
                        # stream (gpsimd is otherwise idle here)
                        rrep = spool.tile([128, SCH], F32, tag="rrep",
                                          name="rrep")
                        nc.gpsimd.partition_broadcast(rrep, rinv)
                        o_h = opool.tile([128, SCH], mm, tag="oh", name="o_h")
                        nc.vector.tensor_mul(o_h, ops, rrep)
                        nc.sync.dma_start(oT_r[:, h, bass.ts(s0, SCH)], o_h)

        # =========== output projection ===========
        with ExitStack() as ph:
            opool = ph.enter_context(tc.tile_pool(name="oT", bufs=1))
            wpool = ph.enter_context(tc.tile_pool(name="wo", bufs=2))
            spool = ph.enter_context(tc.tile_pool(name="oscr", bufs=3))
            pp_mm = ph.enter_context(tc.tile_pool(name="ppmmo", bufs=2,
                                                  space="PSUM"))

            oT_all = opool.tile([128, KO, S_SHARD], mm)
            bo_rep = opool.tile([128, D], mm, name="bo_rep")
            nc.gpsimd.dma_start(bo_rep, bot.ap()[None, :].to_broadcast([128, D]))
            # every engine queue drains at the attention tail, so the O-proj
            # start is gated by how fast the first tiles land AFTER that:
            # 256-col wo tiles (1.3 MB) keep that ramp short.  wo t0 goes
            # first on sync; the oT readbacks stream on scalar.
            wo_sb0 = wpool.tile([128, KO, 256], mm, tag="wo", name="wo_sb")
            nc.sync.dma_start(
                wo_sb0, wo_p.ap()[0].rearrange("p (ko c) -> p ko c", ko=KO))
            for c in range(NSUB):
                if c < NSUB - 1:
                    nc.scalar.dma_start(oT_all[:, :, bass.ts(c, SCH)],
                                        oT_r[:, :, bass.ts(c, SCH)])
                else:
                    csl = bass.ts(c, SCH)
                    nc.sync.dma_start(oT_all[:, :20, csl], oT_r[:, :20, csl])
                    nc.scalar.dma_start(oT_all[:, 20:, csl], oT_r[:, 20:, csl])
            for t in range(D // 256):
                if t == 0:
                    wo_sb = wo_sb0
                else:
                    wo_sb = wpool.tile([128, KO, 256], mm, tag="wo",
                                       name="wo_sb")
                    wdma(t, wo_sb,
                         wo_p.ap()[t].rearrange("p (ko c) -> p ko c", ko=KO))
                for cs in range(S_SHARD // 128):
                    ps = pp_mm.tile([128, 512], F32, tag="mm",
                                    name="ps_o")[:, :256]
                    for ko in range(KO):
                        nc.tensor.matmul(ps, oT_all[:, ko, bass.ts(cs, 128)],
                                         wo_sb[:, ko, :],
                                         start=(ko == 0), stop=(ko == KO - 1))
                    o_sb = spool.tile([128, 512], mm, tag="out",
                                      name="o_sb")[:, :256]
                    nc.vector.tensor_add(o_sb, ps, bo_rep[:, bass.ts(t, 256)])
                    nc.scalar.dma_start(out_r[:, cs, bass.ts(t, 256)], o_sb)

    nc.compile()
    return nc


def _get_nc():
    global _CACHED_NC
    if _CACHED_NC is None:
        _CACHED_NC = _build()
    return _CACHED_NC


def _pack_w(wT, tc):
    """[D, N] (contraction-major transposed weight) -> [N//tc, 128, KO*tc]
    so each streamed tile is one fully-contiguous DMA read."""
    n = wT.shape[1]
    nt = n // tc
    return np.ascontiguousarray(
        wT.reshape(KO, 128, nt, tc).transpose(2, 1, 0, 3).reshape(
            nt, 128, KO * tc))


def kernel(hidden_cond, hidden_uncond, context_cond, context_uncond,
           Wq, bq, Wkv, bkv, gq, gk, Wo, bo):
    global LAST_EXEC_NS
    import ml_dtypes
    bf = ml_dtypes.bfloat16 if MM == mybir.dt.bfloat16 else np.float32
    f32 = np.float32

    nc = _get_nc()

    hid = [np.asarray(hidden_cond, f32).reshape(-1, D),
           np.asarray(hidden_uncond, f32).reshape(-1, D)]
    ctxs = [np.asarray(context_cond, f32).reshape(-1, D),
            np.asarray(context_uncond, f32).reshape(-1, D)]
    Wq = np.asarray(Wq, f32)
    Wkv = np.asarray(Wkv, f32)
    Wo = np.asarray(Wo, f32)
    bq = np.asarray(bq, f32)
    bkv = np.asarray(bkv, f32)
    bo = np.asarray(bo, f32)
    gq = np.asarray(gq, f32)
    gk = np.asarray(gk, f32)
    bk, bv = bkv[:D], bkv[D:]

    wq_pk = _pack_w(np.ascontiguousarray(Wq.T).astype(bf), 128)
    wo_pk = _pack_w(np.ascontiguousarray(Wo.T).astype(bf), 256)
    WkT = np.ascontiguousarray(Wkv[:D].T).astype(bf)
    WvT = np.ascontiguousarray(Wkv[D:].T).astype(bf)
    wk_pks = [_pack_w(WkT[:, r * VSH:(r + 1) * VSH], 256) for r in range(R)]
    wv_pks = [_pack_w(WvT[:, r * VSH:(r + 1) * VSH], 256) for r in range(R)]

    common = {
        "wq_p": wq_pk, "wo_p": wo_pk,
        "gq_pm": np.ascontiguousarray(gq.reshape(KO, 128).T),
        "bqgq_pm": np.ascontiguousarray((bq * gq).reshape(KO, 128).T),
        "bo": bo,
    }
    cT_ps = []
    for g in range(2):
        cT = np.ascontiguousarray(ctxs[g].T).astype(bf)   # [D, L]
        cT_ps.append(np.ascontiguousarray(
            cT.reshape(KO, 128, L).transpose(1, 0, 2).reshape(128, KO * L)))

    in_maps = []
    for core in range(8):
        g, r = core // 4, core % 4
        hT = np.ascontiguousarray(
            hid[g][r * S_SHARD:(r + 1) * S_SHARD].T).astype(bf)  # [D, S_SHARD]
        hT_pk = np.ascontiguousarray(
            hT.reshape(KO, 128, S_SHARD).transpose(1, 0, 2)
            .reshape(128, KO * S_SHARD))
        sl = slice(r * VSH, (r + 1) * VSH)
        in_maps.append({
            "hT_p": hT_pk, "cT_p": cT_ps[g],
            "wk_p": wk_pks[r], "wv_p": wv_pks[r],
            "gk_pm": np.ascontiguousarray(gk[sl].reshape(MSH, 128).T),
            "bkgk_pm": np.ascontiguousarray((bk * gk)[sl].reshape(MSH, 128).T),
            "bv_sh": np.ascontiguousarray(bv[sl]),
            **common,
        })

    res = bass_utils.run_bass_kernel_spmd(nc, in_maps, list(range(8)),
                                          trace=TRACE)
    LAST_EXEC_NS = res.exec_time_ns

    out_c = np.concatenate(
        [np.asarray(res.results[i]["out"], f32) for i in range(4)], axis=0)
    out_u = np.concatenate(
        [np.asarray(res.results[i]["out"], f32) for i in range(4, 8)], axis=0)
    return (out_c[None], out_u[None])
